# revision 15
# baseline (speedup 1.0000x reference)
"""Trainium2 Bass kernel for nn_Discriminator (all-pairs GNN message passing).

Strategy
--------
Data-parallel over batch B=16 across 8 cores (2 batches/core), params replicated.

Algebraic decomposition of layer 1: the all-pairs "relation embedding"
feature matrix [B*N^2, 2F] is never materialized.  For pair row p = i*64+j:
  feature1[p] = input[b, j, :]                      (repeats every 64 rows)
  feature2[p, q] = input[b, i, 8*j + q//64]         (broadcast view)
so   h1pre[p, o] = A[j, o] + sum_k V[p, k] * S[o, k]
with A = input[b] @ W1a.T  (64x512, tiny), V = input[b].reshape(4096, 8),
S[o, k] = sum_t W1b[o, 64k+t].  The A-broadcast rides on DVE as a
broadcast add over each PSUM pair-chunk.

Layers 2/3 (the real work, 68.7 GFLOP total) run as fp32r matmuls (full-rate
at N=512, ~11-bit mantissa) over feature-major activations [512 x 4096] per
batch.  Column chunks are processed in pairs sharing one two-bank PSUM tile,
so every ReLU/bias drain op covers 1024 columns (halving fixed op overhead
on the drain engines, which otherwise rate-limit the PE).  The pair mean
rides the layer-3 ReLU via activation accum_out.  The im/BatchNorm branch is
computed batch-major (N=512 matmuls) then transposed on the PE; its Linear
bias is dropped (cancels exactly in BatchNorm).  The head runs on fp32r
matmuls + DVE reductions.  im_input columns are permuted per-core so each
core's 2 batches land in columns 0:2 (batch-stat invariant).  No collectives.
"""

import numpy as np

import concourse.bass as bass
import concourse.mybir as mybir
import concourse.tile as tile
from concourse import bacc
from concourse.bass_utils import run_bass_kernel_spmd
from concourse.masks import make_identity

B, N, F, IMF = 16, 64, 512, 2048
NPAIR = N * N              # 4096
NCORES = 8
BPC = B // NCORES          # batches per core
P = 128
FC = F // P                # 4 feature chunks
KC_IM = IMF // P           # 16
NPCH = NPAIR // 1024       # 4 paired column chunks of 1024 pairs
EPS = 1e-5

_f32 = mybir.dt.float32
_f32r = mybir.dt.float32r
_Relu = mybir.ActivationFunctionType.Relu
_Sqrt = mybir.ActivationFunctionType.Sqrt
_add = mybir.AluOpType.add
_sub = mybir.AluOpType.subtract
_mult = mybir.AluOpType.mult
_max = mybir.AluOpType.max
_X = mybir.AxisListType.X

_CACHE = {}


def _build_nc():
    nc = bacc.Bacc("TRN2", target_bir_lowering=False, debug=False)

    def din(name, shape, dt=_f32):
        return nc.dram_tensor(name, shape, dt, kind="ExternalInput")

    x_tp = din("x_tp", [BPC, P, FC, N], _f32r)   # input[b].T chunked [kp, kc, j]
    v_tp = din("v_tp", [BPC, NPCH, P, 1024], _f32r)  # V.T zero-padded rows 8:128
    s_t = din("s_t", [P, F], _f32r)              # S.T zero-padded rows 8:128
    w1a = din("w1a", [P, FC, F], _f32r)
    b1 = din("b1", [P, FC])
    w2 = din("w2", [P, FC, F], _f32r)
    b2 = din("b2", [P, FC])
    w3 = din("w3", [P, FC, F], _f32r)
    b3 = din("b3", [P, FC])
    imx = din("imx", [P, KC_IM, B], _f32r)
    imw = din("imw", [P, KC_IM, F], _f32r)
    bng = din("bng", [P, FC])
    bnb = din("bnb", [P, FC])
    d1t = din("d1t", [P, 2 * FC, F], _f32r)
    d1brow = din("d1brow", [BPC, F])             # d1_b replicated on 2 rows
    d2row = din("d2row", [BPC, 2, F])            # d2_w rows replicated on 2 rows
    d2brep = din("d2brep", [BPC, 2])             # d2_b replicated on 2 rows
    out_nt = nc.dram_tensor("out_nt", [BPC, 2], _f32, kind="ExternalOutput")

    with tile.TileContext(nc) as tc:
        with (
            tc.tile_pool(name="singles", bufs=1) as singles,
            tc.tile_pool(name="work", bufs=2) as work,
            tc.tile_pool(name="vpool", bufs=2) as vpool,
            tc.tile_pool(name="chunk", bufs=2) as chunk,
            tc.tile_pool(name="psum", bufs=3, space="PSUM") as psum,
            tc.tile_pool(name="psaux", bufs=1, space="PSUM") as psaux,
        ):
            def load(pool, dram, shape, tag, dt=_f32):
                t = pool.tile(shape, dt, tag=tag)
                nc.sync.dma_start(out=t[:], in_=dram[:])
                return t

            w1a_sb = load(singles, w1a, [P, FC, F], "w1a", _f32r)
            s_sb = load(singles, s_t, [P, F], "s_t", _f32r)
            b1_sb = load(singles, b1, [P, FC], "b1")
            w2_sb = load(singles, w2, [P, FC, F], "w2", _f32r)
            b2_sb = load(singles, b2, [P, FC], "b2")
            w3_sb = load(singles, w3, [P, FC, F], "w3", _f32r)
            b3_sb = load(singles, b3, [P, FC], "b3")
            bng_sb = load(singles, bng, [P, FC], "bng")
            bnb_sb = load(singles, bnb, [P, FC], "bnb")
            d1_sb = load(singles, d1t, [P, 2 * FC, F], "d1t", _f32r)
            d1b_sb = load(singles, d1brow, [BPC, F], "d1brow")
            d2r_sb = load(singles, d2row, [BPC, 2, F], "d2row")
            d2b_sb = load(singles, d2brep, [BPC, 2], "d2brep")

            embT = singles.tile([P, 2 * FC, BPC], _f32r)
            eps_sb = singles.tile([P, 1], _f32)
            nc.vector.memset(eps_sb[:], EPS)
            ident = singles.tile([P, P], _f32)
            make_identity(nc, ident[:])

            # ---- im branch: batch-major matmul, PE transpose, BatchNorm ----
            # transient pool: its SBUF is released before the batch pipeline
            with tc.tile_pool(name="im_pool", bufs=1) as imp:
                imx_sb = load(imp, imx, [P, KC_IM, B], "imx", _f32r)
                imw_sb = load(imp, imw, [P, KC_IM, F], "imw", _f32r)
                pim = psaux.tile([P, 512], _f32, tag="aux")
                for kc in range(KC_IM):
                    nc.tensor.matmul(
                        pim[:B, :], lhsT=imx_sb[:, kc, :], rhs=imw_sb[:, kc, :],
                        start=(kc == 0), stop=(kc == KC_IM - 1),
                    )
                img_nt = imp.tile([P, F], _f32, tag="img_nt")
                nc.gpsimd.memset(img_nt[:], 0.0)
                nc.vector.tensor_copy(out=img_nt[:B, :], in_=pim[:B, :])
                img = imp.tile([P, FC, B], _f32, tag="img")
                for oc in range(FC):
                    ocol = slice(oc * P, (oc + 1) * P)
                    pt = psaux.tile([P, 512], _f32, tag="aux")
                    nc.tensor.transpose(pt[:, :P], img_nt[:, ocol], ident[:])
                    nc.vector.tensor_copy(out=img[:, oc, :], in_=pt[:, :B])
                stats = imp.tile([P, FC, 6], _f32, tag="stats")
                mv = imp.tile([P, FC, 2], _f32, tag="mv")
                rstd = imp.tile([P, FC], _f32, tag="rstd")
                for oc in range(FC):
                    nc.vector.bn_stats(out=stats[:, oc, :], in_=img[:, oc, :])
                    nc.vector.bn_aggr(out=mv[:, oc, :], in_=stats[:, oc, :])
                    nc.scalar.activation(
                        out=rstd[:, oc:oc + 1], in_=mv[:, oc, 1:2], func=_Sqrt,
                        bias=eps_sb[:, 0:1],
                    )
                    nc.vector.reciprocal(
                        out=rstd[:, oc:oc + 1], in_=rstd[:, oc:oc + 1])
                    nc.vector.tensor_scalar(
                        out=img[:, oc, :], in0=img[:, oc, :],
                        scalar1=mv[:, oc, 0:1], scalar2=rstd[:, oc:oc + 1],
                        op0=_sub, op1=_mult,
                    )
                    # relu(gamma * x + beta)
                    nc.scalar.activation(
                        out=img[:, oc, :], in_=img[:, oc, :], func=_Relu,
                        bias=bnb_sb[:, oc:oc + 1], scale=bng_sb[:, oc:oc + 1],
                    )
                nc.vector.tensor_copy(
                    out=embT[:, FC:2 * FC, :], in_=img[:, :, 0:BPC])

            for b in range(BPC):
                x_sb = work.tile([P, FC, N], _f32r, tag="x")
                nc.sync.dma_start(out=x_sb[:], in_=x_tp[b])
                v_sb = vpool.tile([P, NPCH, 1024], _f32r, tag="v", name=f"v{b}")
                for q in range(NPCH):
                    nc.sync.dma_start(out=v_sb[:, q, :], in_=v_tp[b, q])

                # a_t [128, mc, 64] = (input[b] @ W1a.T).T chunks (o on partitions)
                a_t = work.tile([P, FC, N], _f32, tag="a_t")
                for mc in range(FC):
                    mcol = slice(mc * P, (mc + 1) * P)
                    pa = psaux.tile([P, 512], _f32, tag="aux")
                    for kc in range(FC):
                        nc.tensor.matmul(
                            pa[:, :N], lhsT=w1a_sb[:, kc, mcol], rhs=x_sb[:, kc, :],
                            start=(kc == 0), stop=(kc == FC - 1),
                        )
                    nc.vector.tensor_copy(out=a_t[:, mc, :], in_=pa[:, :N])

                acc = work.tile([P, FC, NPCH], _f32, tag="acc")

                for q in range(NPCH):
                    v_q = v_sb[:, q, :]
                    h1 = chunk.tile([P, FC, 1024], _f32r, tag="h1")
                    for mc in range(FC):
                        mcol = slice(mc * P, (mc + 1) * P)
                        p1 = psum.tile([P, 2, 512], _f32, tag="ps")
                        for hf in range(2):
                            nc.tensor.matmul(
                                p1[:, hf, :], lhsT=s_sb[:, mcol],
                                rhs=v_q[:, hf * 512:(hf + 1) * 512],
                                start=True, stop=True,
                            )
                        # + A.T broadcast over the 16 i-blocks (DVE)
                        nc.vector.tensor_tensor(
                            out=h1[:, mc].rearrange("p (m j) -> p m j", j=N),
                            in0=p1[:].rearrange("p h (m j) -> p (h m) j", j=N),
                            in1=a_t[:, mc, None, :].to_broadcast((P, 16, N)),
                            op=_add,
                        )
                        # + b1, relu (DVE)
                        nc.vector.tensor_scalar(
                            out=h1[:, mc], in0=h1[:, mc],
                            scalar1=b1_sb[:, mc:mc + 1], scalar2=0.0,
                            op0=_add, op1=_max,
                        )
                    h2 = chunk.tile([P, FC, 1024], _f32r, tag="h2")
                    for mc in range(FC):
                        mcol = slice(mc * P, (mc + 1) * P)
                        p2 = psum.tile([P, 2, 512], _f32, tag="ps")
                        for hf in range(2):
                            hcol = slice(hf * 512, (hf + 1) * 512)
                            for kc in range(FC):
                                nc.tensor.matmul(
                                    p2[:, hf, :], lhsT=w2_sb[:, kc, mcol],
                                    rhs=h1[:, kc, hcol],
                                    start=(kc == 0), stop=(kc == FC - 1),
                                )
                        nc.scalar.activation(
                            out=h2[:, mc, :],
                            in_=p2[:].rearrange("p h n -> p (h n)"), func=_Relu,
                            bias=b2_sb[:, mc:mc + 1],
                        )
                    for mc in range(FC):
                        mcol = slice(mc * P, (mc + 1) * P)
                        p3 = psum.tile([P, 2, 512], _f32, tag="ps")
                        for hf in range(2):
                            hcol = slice(hf * 512, (hf + 1) * 512)
                            for kc in range(FC):
                                nc.tensor.matmul(
                                    p3[:, hf, :], lhsT=w3_sb[:, kc, mcol],
                                    rhs=h2[:, kc, hcol],
                                    start=(kc == 0), stop=(kc == FC - 1),
                                )
                        h3s = chunk.tile([P, 1024], _f32, tag="h3s")
                        nc.scalar.activation(
                            out=h3s[:],
                            in_=p3[:].rearrange("p h n -> p (h n)"), func=_Relu,
                            bias=b3_sb[:, mc:mc + 1],
                            accum_out=acc[:, mc, q:q + 1],
                        )

                with nc.allow_low_precision(reason="pair-mean partials, 12-bit ok"):
                    for mc in range(FC):
                        nc.vector.tensor_reduce(
                            out=embT[:, mc, b:b + 1], in_=acc[:, mc, :],
                            axis=_X, op=_add,
                        )

            # mean over pairs
            nc.vector.tensor_scalar_mul(
                out=embT[:, 0:FC, :], in0=embT[:, 0:FC, :], scalar1=1.0 / NPAIR,
            )

            # ---- head (2 rows per core) ----
            pd = psaux.tile([P, 512], _f32, tag="aux")
            for kc in range(2 * FC):
                nc.tensor.matmul(
                    pd[:BPC, :], lhsT=embT[:, kc, :], rhs=d1_sb[:, kc, :],
                    start=(kc == 0), stop=(kc == 2 * FC - 1),
                )
            hd1 = work.tile([BPC, F], _f32, tag="hd1")
            nc.vector.tensor_tensor(out=hd1[:], in0=pd[:BPC, :], in1=d1b_sb[:], op=_add)
            nc.vector.tensor_scalar_max(out=hd1[:], in0=hd1[:], scalar1=0.0)
            scr = work.tile([BPC, F], _f32, tag="scr")
            outs = work.tile([BPC, 2], _f32, tag="outs")
            for o in range(2):
                nc.vector.tensor_tensor(
                    out=scr[:], in0=hd1[:], in1=d2r_sb[:, o, :], op=_mult,
                )
                nc.vector.tensor_reduce(
                    out=outs[:, o:o + 1], in_=scr[:], axis=_X, op=_add,
                )
            nc.vector.tensor_tensor(out=outs[:], in0=outs[:], in1=d2b_sb[:], op=_add)
            nc.sync.dma_start(out=out_nt[:], in_=outs[:])

    nc.finalize()
    return nc


def _round_f32r(x):
    """Round fp32 -> fp32r (1s + 8e + 11m, low 12 mantissa bits dropped, RNE)."""
    u = np.ascontiguousarray(x, np.float32).view(np.uint32).copy()
    u += 0x7FF + ((u >> 12) & 1)
    u &= np.uint32(0xFFFFF000)
    return u.view(np.float32)


def _chunk_w(w):
    """[out_f, in_f] weight -> [128, in_f//128, out_f] (lhsT chunks, contiguous)."""
    in_f = w.shape[1]
    return np.ascontiguousarray(
        np.asarray(w, np.float32).T.reshape(in_f // P, P, -1).transpose(1, 0, 2)
    )


def _chunk_b(v):
    """[out_f] bias -> [128, out_f//128] per-partition layout."""
    return np.ascontiguousarray(np.asarray(v, np.float32).reshape(-1, P).T)


def _prep_inputs(input, im_input, gmlp1_w, gmlp1_b, gmlp2_w, gmlp2_b,
                 gmlp3_w, gmlp3_b, im_w, im_b, bn_gamma, bn_beta,
                 d1_w, d1_b, d2_w, d2_b):
    input = np.asarray(input, np.float32)
    im_input = np.asarray(im_input, np.float32)

    S = np.asarray(gmlp1_w)[:, F:].reshape(F, 8, N).sum(-1)   # [512, 8]
    s_t = np.zeros((P, F), np.float32)
    s_t[:8] = S.T

    shared = {
        "s_t": _round_f32r(s_t),
        "w1a": _round_f32r(_chunk_w(np.asarray(gmlp1_w)[:, :F])),
        "b1": _chunk_b(gmlp1_b),
        "w2": _round_f32r(_chunk_w(gmlp2_w)),
        "b2": _chunk_b(gmlp2_b),
        "w3": _round_f32r(_chunk_w(gmlp3_w)),
        "b3": _chunk_b(gmlp3_b),
        "imw": _round_f32r(_chunk_w(im_w)),
        "bng": _chunk_b(bn_gamma),
        "bnb": _chunk_b(bn_beta),
        "d1t": _round_f32r(_chunk_w(d1_w)),
        "d1brow": np.broadcast_to(np.asarray(d1_b, np.float32), (BPC, F)).copy(),
        "d2row": np.broadcast_to(np.asarray(d2_w, np.float32)[None], (BPC, 2, F)).copy(),
        "d2brep": np.broadcast_to(np.asarray(d2_b, np.float32), (BPC, 2)).copy(),
    }

    in_maps = []
    for c in range(NCORES):
        my = [2 * c, 2 * c + 1]
        x_tp = np.zeros((BPC, P, FC, N), np.float32)
        v_tp = np.zeros((BPC, NPCH, P, 1024), np.float32)
        for b in range(BPC):
            xb = input[my[b]]                                   # [64, 512]
            x_tp[b] = xb.T.reshape(FC, P, N).transpose(1, 0, 2)
            vt = xb.reshape(NPAIR, 8).T                         # [8, 4096]
            v_tp[b, :, :8, :] = vt.reshape(8, NPCH, 1024).transpose(1, 0, 2)
        perm = my + [i for i in range(B) if i not in my]
        imx = np.ascontiguousarray(
            im_input[perm].T.reshape(KC_IM, P, B).transpose(1, 0, 2)
        )
        m = dict(shared)
        m["x_tp"] = _round_f32r(x_tp)
        m["v_tp"] = _round_f32r(v_tp)
        m["imx"] = _round_f32r(imx)
        in_maps.append(m)
    return in_maps


def _run(in_maps, **kw):
    if "nc" not in _CACHE:
        _CACHE["nc"] = _build_nc()
    return run_bass_kernel_spmd(_CACHE["nc"], in_maps, core_ids=list(range(NCORES)), **kw)


def kernel(**inputs):
    in_maps = _prep_inputs(**inputs)
    res = _run(in_maps)
    out = np.zeros((B, 2), np.float32)
    for c in range(NCORES):
        out[2 * c:2 * c + 2, :] = res.results[c]["out_nt"]
    return out


# revision 16
# speedup vs baseline: 1.1878x; 1.1878x over previous
"""Trainium2 Bass kernel for nn_Discriminator (all-pairs GNN message passing).

Strategy
--------
Data-parallel over batch B=16 across 8 cores (2 batches/core), params replicated.

Algebraic decomposition of layer 1: the all-pairs "relation embedding"
feature matrix [B*N^2, 2F] is never materialized.  For pair row p = i*64+j:
  feature1[p] = input[b, j, :]                      (repeats every 64 rows)
  feature2[p, q] = input[b, i, 8*j + q//64]         (broadcast view)
so   h1pre[p, o] = A[j, o] + sum_k V[p, k] * S[o, k]
with A = input[b] @ W1a.T  (64x512, tiny), V = input[b].reshape(4096, 8),
S[o, k] = sum_t W1b[o, 64k+t].  The A-broadcast rides on DVE as a
broadcast add over each PSUM chunk.

Layers 2/3 (the real work, 68.7 GFLOP total) run as fp32r matmuls (full-rate
at N=512, ~11-bit mantissa) over feature-major activations [512 x 4096] per
batch.  PSUM drains are split across engines (L1+L2 on DVE, L3 on ACT) so
neither elementwise engine rate-limits the PE.  The pair mean rides the
layer-3 ReLU via activation accum_out.  DMAs are emitted in consumption
order (batch-0 activations first, head weights last) to shorten the cold
start.  The im/BatchNorm branch is computed batch-major (N=512 matmuls) then
transposed on the PE; its Linear bias is dropped (cancels exactly in
BatchNorm); it is emitted between the two batch pipelines so its work fills
PE gaps.  The head runs on fp32r matmuls + DVE reductions.  im_input columns
are permuted per-core so each core's 2 batches land in columns 0:2
(batch-stat invariant).  No collectives.
"""

import numpy as np

import concourse.bass as bass
import concourse.mybir as mybir
import concourse.tile as tile
from concourse import bacc
from concourse.bass_utils import run_bass_kernel_spmd
from concourse.masks import make_identity

B, N, F, IMF = 16, 64, 512, 2048
NPAIR = N * N              # 4096
NCORES = 8
BPC = B // NCORES          # batches per core
P = 128
FC = F // P                # 4 feature chunks
KC_IM = IMF // P           # 16
NCH = NPAIR // 512         # 8 column chunks of 512 pairs
EPS = 1e-5

_f32 = mybir.dt.float32
_f32r = mybir.dt.float32r
_Relu = mybir.ActivationFunctionType.Relu
_Sqrt = mybir.ActivationFunctionType.Sqrt
_add = mybir.AluOpType.add
_sub = mybir.AluOpType.subtract
_mult = mybir.AluOpType.mult
_max = mybir.AluOpType.max
_X = mybir.AxisListType.X

_CACHE = {}


def _build_nc():
    nc = bacc.Bacc("TRN2", target_bir_lowering=False, debug=False)

    def din(name, shape, dt=_f32):
        return nc.dram_tensor(name, shape, dt, kind="ExternalInput")

    x_tp = din("x_tp", [BPC, P, FC, N], _f32r)   # input[b].T chunked [kp, kc, j]
    v_tp = din("v_tp", [BPC, P, NPAIR], _f32r)   # V.T zero-padded rows 8:128
    s_t = din("s_t", [P, F], _f32r)              # S.T zero-padded rows 8:128
    w1a = din("w1a", [P, FC, F], _f32r)
    b1 = din("b1", [P, FC])
    w2 = din("w2", [P, FC, F], _f32r)
    b2 = din("b2", [P, FC])
    w3 = din("w3", [P, FC, F], _f32r)
    b3 = din("b3", [P, FC])
    imx = din("imx", [P, KC_IM, B], _f32r)
    imw = din("imw", [P, KC_IM, F], _f32r)
    bng = din("bng", [P, FC])
    bnb = din("bnb", [P, FC])
    d1t = din("d1t", [P, 2 * FC, F], _f32r)
    d1brow = din("d1brow", [BPC, F])             # d1_b replicated on 2 rows
    d2row = din("d2row", [BPC, 2, F])            # d2_w rows replicated on 2 rows
    d2brep = din("d2brep", [BPC, 2])             # d2_b replicated on 2 rows
    out_nt = nc.dram_tensor("out_nt", [BPC, 2], _f32, kind="ExternalOutput")

    with tile.TileContext(nc) as tc:
        with (
            tc.tile_pool(name="singles", bufs=1) as singles,
            tc.tile_pool(name="work", bufs=2) as work,
            tc.tile_pool(name="chunk", bufs=3) as chunk,
            tc.tile_pool(name="psum", bufs=7, space="PSUM") as psum,
            tc.tile_pool(name="psaux", bufs=1, space="PSUM") as psaux,
        ):
            def load(pool, dram, shape, tag, dt=_f32):
                t = pool.tile(shape, dt, tag=tag)
                nc.sync.dma_start(out=t[:], in_=dram[:])
                return t

            # batch-0 activations first: they gate the cold start
            x_sbs, v_sbs = [], []
            x_0 = work.tile([P, FC, N], _f32r, tag="x", name="x_0")
            nc.sync.dma_start(out=x_0[:], in_=x_tp[0])
            v_0 = work.tile([P, NPAIR], _f32r, tag="v", name="v_0")
            nc.sync.dma_start(out=v_0[:], in_=v_tp[0])
            x_sbs.append(x_0)
            v_sbs.append(v_0)

            w1a_sb = load(singles, w1a, [P, FC, F], "w1a", _f32r)
            s_sb = load(singles, s_t, [P, F], "s_t", _f32r)
            b1_sb = load(singles, b1, [P, FC], "b1")
            w2_sb = load(singles, w2, [P, FC, F], "w2", _f32r)
            b2_sb = load(singles, b2, [P, FC], "b2")
            w3_sb = load(singles, w3, [P, FC, F], "w3", _f32r)
            b3_sb = load(singles, b3, [P, FC], "b3")

            # batch-1 activations: queue behind the hot weights
            x_1 = work.tile([P, FC, N], _f32r, tag="x", name="x_1")
            nc.sync.dma_start(out=x_1[:], in_=x_tp[1])
            v_1 = work.tile([P, NPAIR], _f32r, tag="v", name="v_1")
            nc.sync.dma_start(out=v_1[:], in_=v_tp[1])
            x_sbs.append(x_1)
            v_sbs.append(v_1)

            bng_sb = load(singles, bng, [P, FC], "bng")
            bnb_sb = load(singles, bnb, [P, FC], "bnb")
            d1b_sb = load(singles, d1brow, [BPC, F], "d1brow")
            d2r_sb = load(singles, d2row, [BPC, 2, F], "d2row")
            d2b_sb = load(singles, d2brep, [BPC, 2], "d2brep")

            embT = singles.tile([P, 2 * FC, BPC], _f32r)
            eps_sb = singles.tile([P, 1], _f32)
            nc.vector.memset(eps_sb[:], EPS)
            ident = singles.tile([P, P], _f32)
            make_identity(nc, ident[:])

            def emit_batch(b):
                x_sb, v_sb = x_sbs[b], v_sbs[b]
                # a_t [128, mc, 64] = (input[b] @ W1a.T).T chunks (o on partitions)
                a_t = work.tile([P, FC, N], _f32, tag="a_t", name=f"a_t{b}")
                for mc in range(FC):
                    mcol = slice(mc * P, (mc + 1) * P)
                    pa = psaux.tile([P, 512], _f32, tag="aux")
                    for kc in range(FC):
                        nc.tensor.matmul(
                            pa[:, :N], lhsT=w1a_sb[:, kc, mcol], rhs=x_sb[:, kc, :],
                            start=(kc == 0), stop=(kc == FC - 1),
                        )
                    nc.vector.tensor_copy(out=a_t[:, mc, :], in_=pa[:, :N])

                acc = work.tile([P, FC, NCH], _f32, tag="acc", name=f"acc{b}")

                for n in range(NCH):
                    ncol = slice(n * 512, (n + 1) * 512)
                    h1 = chunk.tile([P, FC, 512], _f32r, tag="h1")
                    for mc in range(FC):
                        mcol = slice(mc * P, (mc + 1) * P)
                        p1 = psum.tile([P, 512], _f32, tag="ps")
                        nc.tensor.matmul(
                            p1[:], lhsT=s_sb[:, mcol], rhs=v_sb[:, ncol],
                            start=True, stop=True,
                        )
                        # + A.T broadcast over the 8 i-blocks (DVE)
                        nc.vector.tensor_tensor(
                            out=h1[:, mc].rearrange("p (m j) -> p m j", j=N),
                            in0=p1[:].rearrange("p (m j) -> p m j", j=N),
                            in1=a_t[:, mc, None, :].to_broadcast((P, 8, N)),
                            op=_add,
                        )
                        # + b1, relu (DVE)
                        nc.vector.tensor_scalar(
                            out=h1[:, mc], in0=h1[:, mc],
                            scalar1=b1_sb[:, mc:mc + 1], scalar2=0.0,
                            op0=_add, op1=_max,
                        )
                    h2 = chunk.tile([P, FC, 512], _f32r, tag="h2")
                    for mc in range(FC):
                        mcol = slice(mc * P, (mc + 1) * P)
                        p2 = psum.tile([P, 512], _f32, tag="ps")
                        for kc in range(FC):
                            nc.tensor.matmul(
                                p2[:], lhsT=w2_sb[:, kc, mcol], rhs=h1[:, kc, :],
                                start=(kc == 0), stop=(kc == FC - 1),
                            )
                        # bias+relu on DVE (balance drains across engines)
                        nc.vector.tensor_scalar(
                            out=h2[:, mc, :], in0=p2[:],
                            scalar1=b2_sb[:, mc:mc + 1], scalar2=0.0,
                            op0=_add, op1=_max,
                        )
                    for mc in range(FC):
                        mcol = slice(mc * P, (mc + 1) * P)
                        p3 = psum.tile([P, 512], _f32, tag="ps")
                        for kc in range(FC):
                            nc.tensor.matmul(
                                p3[:], lhsT=w3_sb[:, kc, mcol], rhs=h2[:, kc, :],
                                start=(kc == 0), stop=(kc == FC - 1),
                            )
                        h3s = chunk.tile([P, 512], _f32, tag="h3s")
                        nc.scalar.activation(
                            out=h3s[:], in_=p3[:], func=_Relu,
                            bias=b3_sb[:, mc:mc + 1],
                            accum_out=acc[:, mc, n:n + 1],
                        )

                with nc.allow_low_precision(reason="pair-mean partials, 12-bit ok"):
                    for mc in range(FC):
                        nc.vector.tensor_reduce(
                            out=embT[:, mc, b:b + 1], in_=acc[:, mc, :],
                            axis=_X, op=_add,
                        )

            emit_batch(0)

            # ---- im branch: batch-major matmul, PE transpose, BatchNorm ----
            # emitted between batches: its DMAs queue after batch-1 activations
            # and its PE/DVE work fills pipeline gaps.
            with tc.tile_pool(name="im_pool", bufs=1) as imp:
                imx_sb = load(imp, imx, [P, KC_IM, B], "imx", _f32r)
                imw_sb = load(imp, imw, [P, KC_IM, F], "imw", _f32r)
                pim = psaux.tile([P, 512], _f32, tag="aux")
                for kc in range(KC_IM):
                    nc.tensor.matmul(
                        pim[:B, :], lhsT=imx_sb[:, kc, :], rhs=imw_sb[:, kc, :],
                        start=(kc == 0), stop=(kc == KC_IM - 1),
                    )
                img_nt = imp.tile([P, F], _f32, tag="img_nt")
                nc.gpsimd.memset(img_nt[:], 0.0)
                nc.vector.tensor_copy(out=img_nt[:B, :], in_=pim[:B, :])
                img = imp.tile([P, FC, B], _f32, tag="img")
                for oc in range(FC):
                    ocol = slice(oc * P, (oc + 1) * P)
                    pt = psaux.tile([P, 512], _f32, tag="aux")
                    nc.tensor.transpose(pt[:, :P], img_nt[:, ocol], ident[:])
                    nc.vector.tensor_copy(out=img[:, oc, :], in_=pt[:, :B])
                stats = imp.tile([P, FC, 6], _f32, tag="stats")
                mv = imp.tile([P, FC, 2], _f32, tag="mv")
                rstd = imp.tile([P, FC], _f32, tag="rstd")
                for oc in range(FC):
                    nc.vector.bn_stats(out=stats[:, oc, :], in_=img[:, oc, :])
                    nc.vector.bn_aggr(out=mv[:, oc, :], in_=stats[:, oc, :])
                    nc.scalar.activation(
                        out=rstd[:, oc:oc + 1], in_=mv[:, oc, 1:2], func=_Sqrt,
                        bias=eps_sb[:, 0:1],
                    )
                    nc.vector.reciprocal(
                        out=rstd[:, oc:oc + 1], in_=rstd[:, oc:oc + 1])
                    nc.vector.tensor_scalar(
                        out=img[:, oc, :], in0=img[:, oc, :],
                        scalar1=mv[:, oc, 0:1], scalar2=rstd[:, oc:oc + 1],
                        op0=_sub, op1=_mult,
                    )
                    # relu(gamma * x + beta)
                    nc.scalar.activation(
                        out=img[:, oc, :], in_=img[:, oc, :], func=_Relu,
                        bias=bnb_sb[:, oc:oc + 1], scale=bng_sb[:, oc:oc + 1],
                    )
                nc.vector.tensor_copy(
                    out=embT[:, FC:2 * FC, :], in_=img[:, :, 0:BPC])

            # head weights: needed last, queued after everything hot
            d1_sb = load(singles, d1t, [P, 2 * FC, F], "d1t", _f32r)

            emit_batch(1)

            # mean over pairs
            nc.vector.tensor_scalar_mul(
                out=embT[:, 0:FC, :], in0=embT[:, 0:FC, :], scalar1=1.0 / NPAIR,
            )

            # ---- head (2 rows per core) ----
            pd = psaux.tile([P, 512], _f32, tag="aux")
            for kc in range(2 * FC):
                nc.tensor.matmul(
                    pd[:BPC, :], lhsT=embT[:, kc, :], rhs=d1_sb[:, kc, :],
                    start=(kc == 0), stop=(kc == 2 * FC - 1),
                )
            hd1 = work.tile([BPC, F], _f32, tag="hd1")
            nc.vector.tensor_tensor(out=hd1[:], in0=pd[:BPC, :], in1=d1b_sb[:], op=_add)
            nc.vector.tensor_scalar_max(out=hd1[:], in0=hd1[:], scalar1=0.0)
            scr = work.tile([BPC, F], _f32, tag="scr")
            outs = work.tile([BPC, 2], _f32, tag="outs")
            for o in range(2):
                nc.vector.tensor_tensor(
                    out=scr[:], in0=hd1[:], in1=d2r_sb[:, o, :], op=_mult,
                )
                nc.vector.tensor_reduce(
                    out=outs[:, o:o + 1], in_=scr[:], axis=_X, op=_add,
                )
            nc.vector.tensor_tensor(out=outs[:], in0=outs[:], in1=d2b_sb[:], op=_add)
            nc.sync.dma_start(out=out_nt[:], in_=outs[:])

    nc.finalize()
    return nc


def _round_f32r(x):
    """Round fp32 -> fp32r (1s + 8e + 11m, low 12 mantissa bits dropped, RNE)."""
    u = np.ascontiguousarray(x, np.float32).view(np.uint32).copy()
    u += 0x7FF + ((u >> 12) & 1)
    u &= np.uint32(0xFFFFF000)
    return u.view(np.float32)


def _chunk_w(w):
    """[out_f, in_f] weight -> [128, in_f//128, out_f] (lhsT chunks, contiguous)."""
    in_f = w.shape[1]
    return np.ascontiguousarray(
        np.asarray(w, np.float32).T.reshape(in_f // P, P, -1).transpose(1, 0, 2)
    )


def _chunk_b(v):
    """[out_f] bias -> [128, out_f//128] per-partition layout."""
    return np.ascontiguousarray(np.asarray(v, np.float32).reshape(-1, P).T)


def _prep_inputs(input, im_input, gmlp1_w, gmlp1_b, gmlp2_w, gmlp2_b,
                 gmlp3_w, gmlp3_b, im_w, im_b, bn_gamma, bn_beta,
                 d1_w, d1_b, d2_w, d2_b):
    input = np.asarray(input, np.float32)
    im_input = np.asarray(im_input, np.float32)

    S = np.asarray(gmlp1_w)[:, F:].reshape(F, 8, N).sum(-1)   # [512, 8]
    s_t = np.zeros((P, F), np.float32)
    s_t[:8] = S.T

    shared = {
        "s_t": _round_f32r(s_t),
        "w1a": _round_f32r(_chunk_w(np.asarray(gmlp1_w)[:, :F])),
        "b1": _chunk_b(gmlp1_b),
        "w2": _round_f32r(_chunk_w(gmlp2_w)),
        "b2": _chunk_b(gmlp2_b),
        "w3": _round_f32r(_chunk_w(gmlp3_w)),
        "b3": _chunk_b(gmlp3_b),
        "imw": _round_f32r(_chunk_w(im_w)),
        "bng": _chunk_b(bn_gamma),
        "bnb": _chunk_b(bn_beta),
        "d1t": _round_f32r(_chunk_w(d1_w)),
        "d1brow": np.broadcast_to(np.asarray(d1_b, np.float32), (BPC, F)).copy(),
        "d2row": np.broadcast_to(np.asarray(d2_w, np.float32)[None], (BPC, 2, F)).copy(),
        "d2brep": np.broadcast_to(np.asarray(d2_b, np.float32), (BPC, 2)).copy(),
    }

    in_maps = []
    for c in range(NCORES):
        my = [2 * c, 2 * c + 1]
        x_tp = np.zeros((BPC, P, FC, N), np.float32)
        v_tp = np.zeros((BPC, P, NPAIR), np.float32)
        for b in range(BPC):
            xb = input[my[b]]                                   # [64, 512]
            x_tp[b] = xb.T.reshape(FC, P, N).transpose(1, 0, 2)
            v_tp[b, :8, :] = xb.reshape(NPAIR, 8).T
        perm = my + [i for i in range(B) if i not in my]
        imx = np.ascontiguousarray(
            im_input[perm].T.reshape(KC_IM, P, B).transpose(1, 0, 2)
        )
        m = dict(shared)
        m["x_tp"] = _round_f32r(x_tp)
        m["v_tp"] = _round_f32r(v_tp)
        m["imx"] = _round_f32r(imx)
        in_maps.append(m)
    return in_maps


def _run(in_maps, **kw):
    if "nc" not in _CACHE:
        _CACHE["nc"] = _build_nc()
    return run_bass_kernel_spmd(_CACHE["nc"], in_maps, core_ids=list(range(NCORES)), **kw)


def kernel(**inputs):
    in_maps = _prep_inputs(**inputs)
    res = _run(in_maps)
    out = np.zeros((B, 2), np.float32)
    for c in range(NCORES):
        out[2 * c:2 * c + 2, :] = res.results[c]["out_nt"]
    return out


# revision 17
# speedup vs baseline: 1.2080x; 1.0170x over previous
"""Trainium2 Bass kernel for nn_Discriminator (all-pairs GNN message passing).

Strategy
--------
Data-parallel over batch B=16 across 8 cores (2 batches/core), params replicated.

Algebraic decomposition of layer 1: the all-pairs "relation embedding"
feature matrix [B*N^2, 2F] is never materialized.  For pair row p = i*64+j:
  feature1[p] = input[b, j, :]                      (repeats every 64 rows)
  feature2[p, q] = input[b, i, 8*j + q//64]         (broadcast view)
so   h1pre[p, o] = A[j, o] + sum_k V[p, k] * S[o, k]
with A = input[b] @ W1a.T  (64x512, tiny), V = input[b].reshape(4096, 8),
S[o, k] = sum_t W1b[o, 64k+t].  The A-broadcast rides on DVE as a
broadcast add over each PSUM chunk.

Layers 2/3 (the real work, 68.7 GFLOP total) run as fp32r matmuls (full-rate
at N=512, ~11-bit mantissa) over feature-major activations [512 x 4096] per
batch.  PSUM drains are split across engines (L1+L2 on DVE, L3 on ACT) so
neither elementwise engine rate-limits the PE.  The pair mean rides the
layer-3 ReLU via activation accum_out.  DMAs are emitted in consumption
order (batch-0 activations first, head weights last) to shorten the cold
start.  The im/BatchNorm branch is computed batch-major (N=512 matmuls) then
transposed on the PE; its Linear bias is dropped (cancels exactly in
BatchNorm); it is emitted between the two batch pipelines so its work fills
PE gaps.  The head runs on fp32r matmuls + DVE reductions.  im_input columns
are permuted per-core so each core's 2 batches land in columns 0:2
(batch-stat invariant).  No collectives.
"""

import numpy as np

import concourse.bass as bass
import concourse.mybir as mybir
import concourse.tile as tile
from concourse import bacc
from concourse.bass_utils import run_bass_kernel_spmd
from concourse.masks import make_identity

B, N, F, IMF = 16, 64, 512, 2048
NPAIR = N * N              # 4096
NCORES = 8
BPC = B // NCORES          # batches per core
P = 128
FC = F // P                # 4 feature chunks
KC_IM = IMF // P           # 16
NCH = NPAIR // 512         # 8 column chunks of 512 pairs
EPS = 1e-5

_f32 = mybir.dt.float32
_f32r = mybir.dt.float32r
_Relu = mybir.ActivationFunctionType.Relu
_Sqrt = mybir.ActivationFunctionType.Sqrt
_add = mybir.AluOpType.add
_sub = mybir.AluOpType.subtract
_mult = mybir.AluOpType.mult
_max = mybir.AluOpType.max
_X = mybir.AxisListType.X

_CACHE = {}


def _build_nc():
    nc = bacc.Bacc("TRN2", target_bir_lowering=False, debug=False)

    def din(name, shape, dt=_f32):
        return nc.dram_tensor(name, shape, dt, kind="ExternalInput")

    x_tp = din("x_tp", [BPC, P, FC, N], _f32r)   # input[b].T chunked [kp, kc, j]
    v_tp = din("v_tp", [BPC, P, NPAIR], _f32r)   # V.T zero-padded rows 8:128
    s_t = din("s_t", [P, F], _f32r)              # S.T zero-padded rows 8:128
    w1a = din("w1a", [P, FC, F], _f32r)
    b1 = din("b1", [P, FC])
    w2 = din("w2", [P, FC, F], _f32r)
    b2 = din("b2", [P, FC])
    w3 = din("w3", [P, FC, F], _f32r)
    b3 = din("b3", [P, FC])
    imx = din("imx", [P, KC_IM, B], _f32r)
    imw = din("imw", [P, KC_IM, F], _f32r)
    bng = din("bng", [P, FC])
    bnb = din("bnb", [P, FC])
    d1t = din("d1t", [P, 2 * FC, F], _f32r)
    d1brow = din("d1brow", [BPC, F])             # d1_b replicated on 2 rows
    d2row = din("d2row", [BPC, 2, F])            # d2_w rows replicated on 2 rows
    d2brep = din("d2brep", [BPC, 2])             # d2_b replicated on 2 rows
    out_nt = nc.dram_tensor("out_nt", [BPC, 2], _f32, kind="ExternalOutput")

    with tile.TileContext(nc) as tc:
        with (
            tc.tile_pool(name="singles", bufs=1) as singles,
            tc.tile_pool(name="work", bufs=2) as work,
            tc.tile_pool(name="chunk", bufs=3) as chunk,
            tc.tile_pool(name="vpool", bufs=2 * NCH) as vpool,
            tc.tile_pool(name="psum", bufs=7, space="PSUM") as psum,
            tc.tile_pool(name="psaux", bufs=1, space="PSUM") as psaux,
        ):
            def load(pool, dram, shape, tag, dt=_f32):
                t = pool.tile(shape, dt, tag=tag)
                nc.sync.dma_start(out=t[:], in_=dram[:])
                return t

            # batch-0 activations first: they gate the cold start
            x_sbs, v_sbs = [], []
            x_0 = work.tile([P, FC, N], _f32r, tag="x", name="x_0")
            nc.sync.dma_start(out=x_0[:], in_=x_tp[0])
            w1a_sb = load(singles, w1a, [P, FC, F], "w1a", _f32r)
            s_sb = load(singles, s_t, [P, F], "s_t", _f32r)
            b1_sb = load(singles, b1, [P, FC], "b1")
            v_0 = []
            for q in range(NCH):
                vq = vpool.tile([P, 512], _f32r, tag="v", name=f"v0_{q}")
                nc.sync.dma_start(out=vq[:], in_=v_tp[0, :, q * 512:(q + 1) * 512])
                v_0.append(vq)
            x_sbs.append(x_0)
            v_sbs.append(v_0)

            w2_sb = load(singles, w2, [P, FC, F], "w2", _f32r)
            b2_sb = load(singles, b2, [P, FC], "b2")
            w3_sb = load(singles, w3, [P, FC, F], "w3", _f32r)
            b3_sb = load(singles, b3, [P, FC], "b3")

            # batch-1 activations: queue behind the hot weights
            x_1 = work.tile([P, FC, N], _f32r, tag="x", name="x_1")
            nc.sync.dma_start(out=x_1[:], in_=x_tp[1])
            v_1 = []
            for q in range(NCH):
                vq = vpool.tile([P, 512], _f32r, tag="v", name=f"v1_{q}")
                nc.sync.dma_start(out=vq[:], in_=v_tp[1, :, q * 512:(q + 1) * 512])
                v_1.append(vq)
            x_sbs.append(x_1)
            v_sbs.append(v_1)

            bng_sb = load(singles, bng, [P, FC], "bng")
            bnb_sb = load(singles, bnb, [P, FC], "bnb")
            d1b_sb = load(singles, d1brow, [BPC, F], "d1brow")
            d2r_sb = load(singles, d2row, [BPC, 2, F], "d2row")
            d2b_sb = load(singles, d2brep, [BPC, 2], "d2brep")

            embT = singles.tile([P, 2 * FC, BPC], _f32r)
            eps_sb = singles.tile([P, 1], _f32)
            nc.vector.memset(eps_sb[:], EPS)
            ident = singles.tile([P, P], _f32)
            make_identity(nc, ident[:])

            def emit_batch(b):
                x_sb, v_sb = x_sbs[b], v_sbs[b]  # v_sb: list of 8 chunk tiles
                # a_t [128, mc, 64] = (input[b] @ W1a.T).T chunks (o on partitions)
                a_t = work.tile([P, FC, N], _f32, tag="a_t", name=f"a_t{b}")
                for mc in range(FC):
                    mcol = slice(mc * P, (mc + 1) * P)
                    pa = psaux.tile([P, 512], _f32, tag="aux")
                    for kc in range(FC):
                        nc.tensor.matmul(
                            pa[:, :N], lhsT=w1a_sb[:, kc, mcol], rhs=x_sb[:, kc, :],
                            start=(kc == 0), stop=(kc == FC - 1),
                        )
                    nc.vector.tensor_scalar_add(
                        out=a_t[:, mc, :], in0=pa[:, :N],
                        scalar1=b1_sb[:, mc:mc + 1])

                acc = work.tile([P, FC, NCH], _f32, tag="acc", name=f"acc{b}")

                for n in range(NCH):
                    h1 = chunk.tile([P, FC, 512], _f32r, tag="h1")
                    for mc in range(FC):
                        mcol = slice(mc * P, (mc + 1) * P)
                        p1 = psum.tile([P, 512], _f32, tag="ps")
                        nc.tensor.matmul(
                            p1[:], lhsT=s_sb[:, mcol], rhs=v_sb[n][:],
                            start=True, stop=True,
                        )
                        # + (A.T + b1) broadcast over the 8 i-blocks (DVE)
                        nc.vector.tensor_tensor(
                            out=h1[:, mc].rearrange("p (m j) -> p m j", j=N),
                            in0=p1[:].rearrange("p (m j) -> p m j", j=N),
                            in1=a_t[:, mc, None, :].to_broadcast((P, 8, N)),
                            op=_add,
                        )
                        # relu (ACT, in place)
                        nc.scalar.activation(
                            out=h1[:, mc], in_=h1[:, mc], func=_Relu,
                        )
                    h2 = chunk.tile([P, FC, 512], _f32r, tag="h2")
                    for mc in range(FC):
                        mcol = slice(mc * P, (mc + 1) * P)
                        p2 = psum.tile([P, 512], _f32, tag="ps")
                        for kc in range(FC):
                            nc.tensor.matmul(
                                p2[:], lhsT=w2_sb[:, kc, mcol], rhs=h1[:, kc, :],
                                start=(kc == 0), stop=(kc == FC - 1),
                            )
                        # bias+relu on DVE (balance drains across engines)
                        nc.vector.tensor_scalar(
                            out=h2[:, mc, :], in0=p2[:],
                            scalar1=b2_sb[:, mc:mc + 1], scalar2=0.0,
                            op0=_add, op1=_max,
                        )
                    for mc in range(FC):
                        mcol = slice(mc * P, (mc + 1) * P)
                        p3 = psum.tile([P, 512], _f32, tag="ps")
                        for kc in range(FC):
                            nc.tensor.matmul(
                                p3[:], lhsT=w3_sb[:, kc, mcol], rhs=h2[:, kc, :],
                                start=(kc == 0), stop=(kc == FC - 1),
                            )
                        h3s = chunk.tile([P, 512], _f32, tag="h3s")
                        nc.scalar.activation(
                            out=h3s[:], in_=p3[:], func=_Relu,
                            bias=b3_sb[:, mc:mc + 1],
                            accum_out=acc[:, mc, n:n + 1],
                        )

                with nc.allow_low_precision(reason="pair-mean partials, 12-bit ok"):
                    for mc in range(FC):
                        nc.vector.tensor_reduce(
                            out=embT[:, mc, b:b + 1], in_=acc[:, mc, :],
                            axis=_X, op=_add,
                        )

            emit_batch(0)

            # ---- im branch: batch-major matmul, PE transpose, BatchNorm ----
            # emitted between batches: its DMAs queue after batch-1 activations
            # and its PE/DVE work fills pipeline gaps.
            with tc.tile_pool(name="im_pool", bufs=1) as imp:
                imx_sb = load(imp, imx, [P, KC_IM, B], "imx", _f32r)
                imw_sb = load(imp, imw, [P, KC_IM, F], "imw", _f32r)
                pim = psaux.tile([P, 512], _f32, tag="aux")
                for kc in range(KC_IM):
                    nc.tensor.matmul(
                        pim[:B, :], lhsT=imx_sb[:, kc, :], rhs=imw_sb[:, kc, :],
                        start=(kc == 0), stop=(kc == KC_IM - 1),
                    )
                img_nt = imp.tile([P, F], _f32, tag="img_nt")
                nc.gpsimd.memset(img_nt[:], 0.0)
                nc.vector.tensor_copy(out=img_nt[:B, :], in_=pim[:B, :])
                img = imp.tile([P, FC, B], _f32, tag="img")
                for oc in range(FC):
                    ocol = slice(oc * P, (oc + 1) * P)
                    pt = psaux.tile([P, 512], _f32, tag="aux")
                    nc.tensor.transpose(pt[:, :P], img_nt[:, ocol], ident[:])
                    nc.vector.tensor_copy(out=img[:, oc, :], in_=pt[:, :B])
                stats = imp.tile([P, FC, 6], _f32, tag="stats")
                mv = imp.tile([P, FC, 2], _f32, tag="mv")
                rstd = imp.tile([P, FC], _f32, tag="rstd")
                for oc in range(FC):
                    nc.vector.bn_stats(out=stats[:, oc, :], in_=img[:, oc, :])
                    nc.vector.bn_aggr(out=mv[:, oc, :], in_=stats[:, oc, :])
                    nc.scalar.activation(
                        out=rstd[:, oc:oc + 1], in_=mv[:, oc, 1:2], func=_Sqrt,
                        bias=eps_sb[:, 0:1],
                    )
                    nc.vector.reciprocal(
                        out=rstd[:, oc:oc + 1], in_=rstd[:, oc:oc + 1])
                    nc.vector.tensor_scalar(
                        out=img[:, oc, :], in0=img[:, oc, :],
                        scalar1=mv[:, oc, 0:1], scalar2=rstd[:, oc:oc + 1],
                        op0=_sub, op1=_mult,
                    )
                    # relu(gamma * x + beta)
                    nc.scalar.activation(
                        out=img[:, oc, :], in_=img[:, oc, :], func=_Relu,
                        bias=bnb_sb[:, oc:oc + 1], scale=bng_sb[:, oc:oc + 1],
                    )
                nc.vector.tensor_copy(
                    out=embT[:, FC:2 * FC, :], in_=img[:, :, 0:BPC])

            # head weights: needed last, queued after everything hot
            d1_sb = load(singles, d1t, [P, 2 * FC, F], "d1t", _f32r)

            emit_batch(1)

            # ---- head (2 rows per core) ----
            pd = psaux.tile([P, 512], _f32, tag="aux")
            kc_order = list(range(FC, 2 * FC)) + list(range(FC))
            for i, kc in enumerate(kc_order):
                nc.tensor.matmul(
                    pd[:BPC, :], lhsT=embT[:, kc, :], rhs=d1_sb[:, kc, :],
                    start=(i == 0), stop=(i == 2 * FC - 1),
                )
            hd1 = work.tile([BPC, F], _f32, tag="hd1")
            nc.vector.tensor_tensor(out=hd1[:], in0=pd[:BPC, :], in1=d1b_sb[:], op=_add)
            nc.vector.tensor_scalar_max(out=hd1[:], in0=hd1[:], scalar1=0.0)
            scr = work.tile([BPC, F], _f32, tag="scr")
            outs = work.tile([BPC, 2], _f32, tag="outs")
            for o in range(2):
                nc.vector.tensor_tensor(
                    out=scr[:], in0=hd1[:], in1=d2r_sb[:, o, :], op=_mult,
                )
                nc.vector.tensor_reduce(
                    out=outs[:, o:o + 1], in_=scr[:], axis=_X, op=_add,
                )
            nc.vector.tensor_tensor(out=outs[:], in0=outs[:], in1=d2b_sb[:], op=_add)
            nc.sync.dma_start(out=out_nt[:], in_=outs[:])

    nc.finalize()
    return nc


def _round_f32r(x):
    """Round fp32 -> fp32r (1s + 8e + 11m, low 12 mantissa bits dropped, RNE)."""
    u = np.ascontiguousarray(x, np.float32).view(np.uint32).copy()
    u += 0x7FF + ((u >> 12) & 1)
    u &= np.uint32(0xFFFFF000)
    return u.view(np.float32)


def _chunk_w(w):
    """[out_f, in_f] weight -> [128, in_f//128, out_f] (lhsT chunks, contiguous)."""
    in_f = w.shape[1]
    return np.ascontiguousarray(
        np.asarray(w, np.float32).T.reshape(in_f // P, P, -1).transpose(1, 0, 2)
    )


def _d1_scaled(d1_w):
    """Fold the 1/NPAIR pair-mean into the sent-half of d1's weights."""
    w = np.asarray(d1_w, np.float32).copy()
    w[:, :F] *= 1.0 / NPAIR
    return _chunk_w(w)


def _chunk_b(v):
    """[out_f] bias -> [128, out_f//128] per-partition layout."""
    return np.ascontiguousarray(np.asarray(v, np.float32).reshape(-1, P).T)


def _prep_inputs(input, im_input, gmlp1_w, gmlp1_b, gmlp2_w, gmlp2_b,
                 gmlp3_w, gmlp3_b, im_w, im_b, bn_gamma, bn_beta,
                 d1_w, d1_b, d2_w, d2_b):
    input = np.asarray(input, np.float32)
    im_input = np.asarray(im_input, np.float32)

    S = np.asarray(gmlp1_w)[:, F:].reshape(F, 8, N).sum(-1)   # [512, 8]
    s_t = np.zeros((P, F), np.float32)
    s_t[:8] = S.T

    shared = {
        "s_t": _round_f32r(s_t),
        "w1a": _round_f32r(_chunk_w(np.asarray(gmlp1_w)[:, :F])),
        "b1": _chunk_b(gmlp1_b),
        "w2": _round_f32r(_chunk_w(gmlp2_w)),
        "b2": _chunk_b(gmlp2_b),
        "w3": _round_f32r(_chunk_w(gmlp3_w)),
        "b3": _chunk_b(gmlp3_b),
        "imw": _round_f32r(_chunk_w(im_w)),
        "bng": _chunk_b(bn_gamma),
        "bnb": _chunk_b(bn_beta),
        "d1t": _round_f32r(_d1_scaled(d1_w)),
        "d1brow": np.broadcast_to(np.asarray(d1_b, np.float32), (BPC, F)).copy(),
        "d2row": np.broadcast_to(np.asarray(d2_w, np.float32)[None], (BPC, 2, F)).copy(),
        "d2brep": np.broadcast_to(np.asarray(d2_b, np.float32), (BPC, 2)).copy(),
    }

    in_maps = []
    for c in range(NCORES):
        my = [2 * c, 2 * c + 1]
        x_tp = np.zeros((BPC, P, FC, N), np.float32)
        v_tp = np.zeros((BPC, P, NPAIR), np.float32)
        for b in range(BPC):
            xb = input[my[b]]                                   # [64, 512]
            x_tp[b] = xb.T.reshape(FC, P, N).transpose(1, 0, 2)
            v_tp[b, :8, :] = xb.reshape(NPAIR, 8).T
        perm = my + [i for i in range(B) if i not in my]
        imx = np.ascontiguousarray(
            im_input[perm].T.reshape(KC_IM, P, B).transpose(1, 0, 2)
        )
        m = dict(shared)
        m["x_tp"] = _round_f32r(x_tp)
        m["v_tp"] = _round_f32r(v_tp)
        m["imx"] = _round_f32r(imx)
        in_maps.append(m)
    return in_maps


def _run(in_maps, **kw):
    if "nc" not in _CACHE:
        _CACHE["nc"] = _build_nc()
    return run_bass_kernel_spmd(_CACHE["nc"], in_maps, core_ids=list(range(NCORES)), **kw)


def kernel(**inputs):
    in_maps = _prep_inputs(**inputs)
    res = _run(in_maps)
    out = np.zeros((B, 2), np.float32)
    for c in range(NCORES):
        out[2 * c:2 * c + 2, :] = res.results[c]["out_nt"]
    return out


# revision 18
# speedup vs baseline: 1.3336x; 1.1040x over previous
"""Trainium2 Bass kernel for nn_Discriminator (all-pairs GNN message passing).

Strategy
--------
Data-parallel over batch B=16 across 8 cores (2 batches/core), params replicated.

Algebraic decomposition of layer 1: the all-pairs "relation embedding"
feature matrix [B*N^2, 2F] is never materialized.  For pair row p = i*64+j:
  feature1[p] = input[b, j, :]                      (repeats every 64 rows)
  feature2[p, q] = input[b, i, 8*j + q//64]         (broadcast view)
so   h1pre[p, o] = A[j, o] + sum_k V[p, k] * S[o, k]
with A = input[b] @ W1a.T  (64x512, tiny), V = input[b].reshape(4096, 8),
S[o, k] = sum_t W1b[o, 64k+t].  The A-broadcast rides on DVE as a
broadcast add over each PSUM chunk.

Layers 2/3 (the real work, 68.7 GFLOP total) run as fp32r matmuls (full-rate
at N=512, ~11-bit mantissa) over feature-major activations [512 x 4096] per
batch.  PSUM drains are split across engines (L1+L2 on DVE, L3 on ACT) so
neither elementwise engine rate-limits the PE.  The pair mean rides the
layer-3 ReLU via activation accum_out.  DMAs are emitted in consumption
order (batch-0 activations first, head weights last) to shorten the cold
start.  The im/BatchNorm branch is computed batch-major (N=512 matmuls) then
transposed on the PE; its Linear bias is dropped (cancels exactly in
BatchNorm); it is emitted between the two batch pipelines so its work fills
PE gaps.  The head runs on fp32r matmuls + DVE reductions.  im_input columns
are permuted per-core so each core's 2 batches land in columns 0:2
(batch-stat invariant).  No collectives.
"""

import numpy as np

import concourse.bass as bass
import concourse.mybir as mybir
import concourse.tile as tile
from concourse import bacc
from concourse.bass_utils import run_bass_kernel_spmd
from concourse.masks import make_identity

B, N, F, IMF = 16, 64, 512, 2048
NPAIR = N * N              # 4096
NCORES = 8
BPC = B // NCORES          # batches per core
P = 128
FC = F // P                # 4 feature chunks
KC_IM = IMF // P           # 16
NCH = NPAIR // 512         # 8 column chunks of 512 pairs
EPS = 1e-5

_f32 = mybir.dt.float32
_f32r = mybir.dt.float32r
_Relu = mybir.ActivationFunctionType.Relu
_Sqrt = mybir.ActivationFunctionType.Sqrt
_add = mybir.AluOpType.add
_sub = mybir.AluOpType.subtract
_mult = mybir.AluOpType.mult
_max = mybir.AluOpType.max
_X = mybir.AxisListType.X

_CACHE = {}


def _build_nc():
    nc = bacc.Bacc("TRN2", target_bir_lowering=False, debug=False)

    def din(name, shape, dt=_f32):
        return nc.dram_tensor(name, shape, dt, kind="ExternalInput")

    x_tp = din("x_tp", [BPC, P, FC, N], _f32r)   # input[b].T chunked [kp, kc, j]
    v_tp = din("v_tp", [BPC, P, NPAIR], _f32r)   # V.T zero-padded rows 8:128
    s_t = din("s_t", [P, F], _f32r)              # S.T zero-padded rows 8:128
    w1a = din("w1a", [P, FC, F], _f32r)
    b1 = din("b1", [P, FC])
    w2 = din("w2", [P, FC, F], _f32r)
    b2 = din("b2", [P, FC])
    w3 = din("w3", [P, FC, F], _f32r)
    b3 = din("b3", [P, FC])
    imx = din("imx", [P, KC_IM, B], _f32r)
    imw = din("imw", [P, KC_IM, F], _f32r)
    bng = din("bng", [P, FC])
    bnb = din("bnb", [P, FC])
    d1t = din("d1t", [P, 2 * FC, F], _f32r)
    d1brow = din("d1brow", [BPC, F])             # d1_b replicated on 2 rows
    d2row = din("d2row", [BPC, 2, F])            # d2_w rows replicated on 2 rows
    d2brep = din("d2brep", [BPC, 2])             # d2_b replicated on 2 rows
    out_nt = nc.dram_tensor("out_nt", [BPC, 2], _f32, kind="ExternalOutput")

    with tile.TileContext(nc) as tc:
        with (
            tc.tile_pool(name="singles", bufs=1) as singles,
            tc.tile_pool(name="work", bufs=2) as work,
            tc.tile_pool(name="chunk", bufs=3) as chunk,
            tc.tile_pool(name="vpool", bufs=2 * NCH) as vpool,
            tc.tile_pool(name="psum", bufs=7, space="PSUM") as psum,
            tc.tile_pool(name="psaux", bufs=1, space="PSUM") as psaux,
        ):
            def load(pool, dram, shape, tag, dt=_f32):
                t = pool.tile(shape, dt, tag=tag)
                nc.sync.dma_start(out=t[:], in_=dram[:])
                return t

            # batch-0 activations first: they gate the cold start
            x_sbs, v_sbs = [], []
            x_0 = work.tile([P, FC, N], _f32r, tag="x", name="x_0")
            nc.sync.dma_start(out=x_0[:], in_=x_tp[0])
            w1a_sb = load(singles, w1a, [P, FC, F], "w1a", _f32r)
            s_sb = load(singles, s_t, [P, F], "s_t", _f32r)
            b1_sb = load(singles, b1, [P, FC], "b1")
            v_0 = []
            def v0_dma(q):
                vq = vpool.tile([P, 512], _f32r, tag="v", name=f"v0_{q}")
                nc.sync.dma_start(out=vq[:], in_=v_tp[0, :, q * 512:(q + 1) * 512])
                v_0.append(vq)
            v0_dma(0)
            v0_dma(1)
            w2_sb = load(singles, w2, [P, FC, F], "w2", _f32r)
            b2_sb = load(singles, b2, [P, FC], "b2")
            v0_dma(2)
            v0_dma(3)
            w3_sb = load(singles, w3, [P, FC, F], "w3", _f32r)
            b3_sb = load(singles, b3, [P, FC], "b3")
            for q in range(4, NCH):
                v0_dma(q)
            x_sbs.append(x_0)
            v_sbs.append(v_0)

            # batch-1 activations: queue behind the hot weights
            x_1 = work.tile([P, FC, N], _f32r, tag="x", name="x_1")
            nc.sync.dma_start(out=x_1[:], in_=x_tp[1])
            v_1 = []
            for q in range(NCH):
                vq = vpool.tile([P, 512], _f32r, tag="v", name=f"v1_{q}")
                nc.sync.dma_start(out=vq[:], in_=v_tp[1, :, q * 512:(q + 1) * 512])
                v_1.append(vq)
            x_sbs.append(x_1)
            v_sbs.append(v_1)

            bng_sb = load(singles, bng, [P, FC], "bng")
            bnb_sb = load(singles, bnb, [P, FC], "bnb")
            d1b_sb = load(singles, d1brow, [BPC, F], "d1brow")
            d2r_sb = load(singles, d2row, [BPC, 2, F], "d2row")
            d2b_sb = load(singles, d2brep, [BPC, 2], "d2brep")

            embT = singles.tile([P, 2 * FC, BPC], _f32r)
            eps_sb = singles.tile([P, 1], _f32)
            nc.vector.memset(eps_sb[:], EPS)
            ident = singles.tile([P, P], _f32)
            make_identity(nc, ident[:])

            def emit_batch(b):
                x_sb, v_sb = x_sbs[b], v_sbs[b]  # v_sb: list of 8 chunk tiles
                # a_t [128, mc, 64] = (input[b] @ W1a.T).T + b1 (o on partitions)
                a_t = work.tile([P, FC, N], _f32, tag="a_t", name=f"a_t{b}")
                for mc in range(FC):
                    mcol = slice(mc * P, (mc + 1) * P)
                    pa = psaux.tile([P, 512], _f32, tag="aux")
                    for kc in range(FC):
                        nc.tensor.matmul(
                            pa[:, :N], lhsT=w1a_sb[:, kc, mcol], rhs=x_sb[:, kc, :],
                            start=(kc == 0), stop=(kc == FC - 1),
                        )
                    nc.vector.tensor_scalar_add(
                        out=a_t[:, mc, :], in0=pa[:, :N],
                        scalar1=b1_sb[:, mc:mc + 1])

                acc = work.tile([P, FC, NCH], _f32, tag="acc", name=f"acc{b}")

                h1s = {}
                h2s = {}

                def stage1(n):
                    h1 = chunk.tile([P, FC, 512], _f32r, tag="h1")
                    h1s[n] = h1
                    for mc in range(FC):
                        mcol = slice(mc * P, (mc + 1) * P)
                        p1 = psum.tile([P, 512], _f32, tag="ps")
                        nc.tensor.matmul(
                            p1[:], lhsT=s_sb[:, mcol], rhs=v_sb[n][:],
                            start=True, stop=True,
                        )
                        # + (A.T + b1) broadcast over the 8 i-blocks (DVE)
                        nc.vector.tensor_tensor(
                            out=h1[:, mc].rearrange("p (m j) -> p m j", j=N),
                            in0=p1[:].rearrange("p (m j) -> p m j", j=N),
                            in1=a_t[:, mc, None, :].to_broadcast((P, 8, N)),
                            op=_add,
                        )
                        # relu (ACT, in place)
                        nc.scalar.activation(
                            out=h1[:, mc], in_=h1[:, mc], func=_Relu,
                        )

                def stage2(n):
                    h1 = h1s.pop(n)
                    h2 = chunk.tile([P, FC, 512], _f32r, tag="h2")
                    h2s[n] = h2
                    for mc in range(FC):
                        mcol = slice(mc * P, (mc + 1) * P)
                        p2 = psum.tile([P, 512], _f32, tag="ps")
                        for kc in range(FC):
                            nc.tensor.matmul(
                                p2[:], lhsT=w2_sb[:, kc, mcol], rhs=h1[:, kc, :],
                                start=(kc == 0), stop=(kc == FC - 1),
                            )
                        # bias+relu on DVE (balance drains across engines)
                        nc.vector.tensor_scalar(
                            out=h2[:, mc, :], in0=p2[:],
                            scalar1=b2_sb[:, mc:mc + 1], scalar2=0.0,
                            op0=_add, op1=_max,
                        )

                def stage3(n):
                    h2 = h2s.pop(n)
                    for mc in range(FC):
                        mcol = slice(mc * P, (mc + 1) * P)
                        p3 = psum.tile([P, 512], _f32, tag="ps")
                        for kc in range(FC):
                            nc.tensor.matmul(
                                p3[:], lhsT=w3_sb[:, kc, mcol], rhs=h2[:, kc, :],
                                start=(kc == 0), stop=(kc == FC - 1),
                            )
                        h3s = chunk.tile([P, 512], _f32, tag="h3s")
                        nc.scalar.activation(
                            out=h3s[:], in_=p3[:], func=_Relu,
                            bias=b3_sb[:, mc:mc + 1],
                            accum_out=acc[:, mc, n:n + 1],
                        )

                # stage-lagged emission: PE never sits behind a chunk's own
                # drain chain (L1 of n+1 is in program order before L2 of n)
                for n in range(NCH + 2):
                    if n < NCH:
                        stage1(n)
                    if 1 <= n and n - 1 < NCH:
                        stage2(n - 1)
                    if 2 <= n:
                        stage3(n - 2)

                with nc.allow_low_precision(reason="pair-mean partials, 12-bit ok"):
                    for mc in range(FC):
                        nc.vector.tensor_reduce(
                            out=embT[:, mc, b:b + 1], in_=acc[:, mc, :],
                            axis=_X, op=_add,
                        )

            emit_batch(0)

            # ---- im branch: batch-major matmul, PE transpose, BatchNorm ----
            # emitted between batches: its DMAs queue after batch-1 activations
            # and its PE/DVE work fills pipeline gaps.
            with tc.tile_pool(name="im_pool", bufs=1) as imp:
                imx_sb = load(imp, imx, [P, KC_IM, B], "imx", _f32r)
                imw_sb = load(imp, imw, [P, KC_IM, F], "imw", _f32r)
                pim = psaux.tile([P, 512], _f32, tag="aux")
                for kc in range(KC_IM):
                    nc.tensor.matmul(
                        pim[:B, :], lhsT=imx_sb[:, kc, :], rhs=imw_sb[:, kc, :],
                        start=(kc == 0), stop=(kc == KC_IM - 1),
                    )
                img_nt = imp.tile([P, F], _f32, tag="img_nt")
                nc.gpsimd.memset(img_nt[:], 0.0)
                nc.vector.tensor_copy(out=img_nt[:B, :], in_=pim[:B, :])
                img = imp.tile([P, FC, B], _f32, tag="img")
                for oc in range(FC):
                    ocol = slice(oc * P, (oc + 1) * P)
                    pt = psaux.tile([P, 512], _f32, tag="aux")
                    nc.tensor.transpose(pt[:, :P], img_nt[:, ocol], ident[:])
                    nc.vector.tensor_copy(out=img[:, oc, :], in_=pt[:, :B])
                stats = imp.tile([P, FC, 6], _f32, tag="stats")
                mv = imp.tile([P, FC, 2], _f32, tag="mv")
                rstd = imp.tile([P, FC], _f32, tag="rstd")
                for oc in range(FC):
                    nc.vector.bn_stats(out=stats[:, oc, :], in_=img[:, oc, :])
                    nc.vector.bn_aggr(out=mv[:, oc, :], in_=stats[:, oc, :])
                    nc.scalar.activation(
                        out=rstd[:, oc:oc + 1], in_=mv[:, oc, 1:2], func=_Sqrt,
                        bias=eps_sb[:, 0:1],
                    )
                    nc.vector.reciprocal(
                        out=rstd[:, oc:oc + 1], in_=rstd[:, oc:oc + 1])
                    nc.vector.tensor_scalar(
                        out=img[:, oc, :], in0=img[:, oc, :],
                        scalar1=mv[:, oc, 0:1], scalar2=rstd[:, oc:oc + 1],
                        op0=_sub, op1=_mult,
                    )
                    # relu(gamma * x + beta)
                    nc.scalar.activation(
                        out=img[:, oc, :], in_=img[:, oc, :], func=_Relu,
                        bias=bnb_sb[:, oc:oc + 1], scale=bng_sb[:, oc:oc + 1],
                    )
                nc.vector.tensor_copy(
                    out=embT[:, FC:2 * FC, :], in_=img[:, :, 0:BPC])

            # head weights: needed last, queued after everything hot
            d1_sb = load(singles, d1t, [P, 2 * FC, F], "d1t", _f32r)

            emit_batch(1)

            # ---- head (2 rows per core) ----
            pd = psaux.tile([P, 512], _f32, tag="aux")
            kc_order = list(range(FC, 2 * FC)) + list(range(FC))
            for i, kc in enumerate(kc_order):
                nc.tensor.matmul(
                    pd[:BPC, :], lhsT=embT[:, kc, :], rhs=d1_sb[:, kc, :],
                    start=(i == 0), stop=(i == 2 * FC - 1),
                )
            hd1 = work.tile([BPC, F], _f32, tag="hd1")
            nc.vector.tensor_tensor(out=hd1[:], in0=pd[:BPC, :], in1=d1b_sb[:], op=_add)
            nc.vector.tensor_scalar_max(out=hd1[:], in0=hd1[:], scalar1=0.0)
            scr = work.tile([BPC, F], _f32, tag="scr")
            outs = work.tile([BPC, 2], _f32, tag="outs")
            for o in range(2):
                nc.vector.tensor_tensor(
                    out=scr[:], in0=hd1[:], in1=d2r_sb[:, o, :], op=_mult,
                )
                nc.vector.tensor_reduce(
                    out=outs[:, o:o + 1], in_=scr[:], axis=_X, op=_add,
                )
            nc.vector.tensor_tensor(out=outs[:], in0=outs[:], in1=d2b_sb[:], op=_add)
            nc.sync.dma_start(out=out_nt[:], in_=outs[:])

    nc.finalize()
    return nc


def _round_f32r(x):
    """Round fp32 -> fp32r (1s + 8e + 11m, low 12 mantissa bits dropped, RNE)."""
    u = np.ascontiguousarray(x, np.float32).view(np.uint32).copy()
    u += 0x7FF + ((u >> 12) & 1)
    u &= np.uint32(0xFFFFF000)
    return u.view(np.float32)


def _chunk_w(w):
    """[out_f, in_f] weight -> [128, in_f//128, out_f] (lhsT chunks, contiguous)."""
    in_f = w.shape[1]
    return np.ascontiguousarray(
        np.asarray(w, np.float32).T.reshape(in_f // P, P, -1).transpose(1, 0, 2)
    )


def _d1_scaled(d1_w):
    """Fold the 1/NPAIR pair-mean into the sent-half of d1's weights."""
    w = np.asarray(d1_w, np.float32).copy()
    w[:, :F] *= 1.0 / NPAIR
    return _chunk_w(w)


def _chunk_b(v):
    """[out_f] bias -> [128, out_f//128] per-partition layout."""
    return np.ascontiguousarray(np.asarray(v, np.float32).reshape(-1, P).T)


def _prep_inputs(input, im_input, gmlp1_w, gmlp1_b, gmlp2_w, gmlp2_b,
                 gmlp3_w, gmlp3_b, im_w, im_b, bn_gamma, bn_beta,
                 d1_w, d1_b, d2_w, d2_b):
    input = np.asarray(input, np.float32)
    im_input = np.asarray(im_input, np.float32)

    S = np.asarray(gmlp1_w)[:, F:].reshape(F, 8, N).sum(-1)   # [512, 8]
    s_t = np.zeros((P, F), np.float32)
    s_t[:8] = S.T

    shared = {
        "s_t": _round_f32r(s_t),
        "w1a": _round_f32r(_chunk_w(np.asarray(gmlp1_w)[:, :F])),
        "b1": _chunk_b(gmlp1_b),
        "w2": _round_f32r(_chunk_w(gmlp2_w)),
        "b2": _chunk_b(gmlp2_b),
        "w3": _round_f32r(_chunk_w(gmlp3_w)),
        "b3": _chunk_b(gmlp3_b),
        "imw": _round_f32r(_chunk_w(im_w)),
        "bng": _chunk_b(bn_gamma),
        "bnb": _chunk_b(bn_beta),
        "d1t": _round_f32r(_d1_scaled(d1_w)),
        "d1brow": np.broadcast_to(np.asarray(d1_b, np.float32), (BPC, F)).copy(),
        "d2row": np.broadcast_to(np.asarray(d2_w, np.float32)[None], (BPC, 2, F)).copy(),
        "d2brep": np.broadcast_to(np.asarray(d2_b, np.float32), (BPC, 2)).copy(),
    }

    in_maps = []
    for c in range(NCORES):
        my = [2 * c, 2 * c + 1]
        x_tp = np.zeros((BPC, P, FC, N), np.float32)
        v_tp = np.zeros((BPC, P, NPAIR), np.float32)
        for b in range(BPC):
            xb = input[my[b]]                                   # [64, 512]
            x_tp[b] = xb.T.reshape(FC, P, N).transpose(1, 0, 2)
            v_tp[b, :8, :] = xb.reshape(NPAIR, 8).T
        perm = my + [i for i in range(B) if i not in my]
        imx = np.ascontiguousarray(
            im_input[perm].T.reshape(KC_IM, P, B).transpose(1, 0, 2)
        )
        m = dict(shared)
        m["x_tp"] = _round_f32r(x_tp)
        m["v_tp"] = _round_f32r(v_tp)
        m["imx"] = _round_f32r(imx)
        in_maps.append(m)
    return in_maps


def _run(in_maps, **kw):
    if "nc" not in _CACHE:
        _CACHE["nc"] = _build_nc()
    return run_bass_kernel_spmd(_CACHE["nc"], in_maps, core_ids=list(range(NCORES)), **kw)


def kernel(**inputs):
    in_maps = _prep_inputs(**inputs)
    res = _run(in_maps)
    out = np.zeros((B, 2), np.float32)
    for c in range(NCORES):
        out[2 * c:2 * c + 2, :] = res.results[c]["out_nt"]
    return out
